# revision 11
# baseline (speedup 1.0000x reference)
"""Trainium2 Bass kernel: masked multi-head attention (B=2, S=2048, D=512, H=8).

Sharding: batch x head-pair across 8 cores (core = b*4 + head_pair).
Each core computes, for its batch b and its 2 heads:
    q/k/v projections -> scores^T -> exp (mask folded in as per-partition
    bias on the ScalarE) -> attn@v with a ones-column appended to V (gives
    the softmax denominator for free) -> normalize -> partial out-proj.
The 4 per-batch partials are summed on the host (the "all-reduce"), then
bias bo is added.

Device layouts (per core):
  xTq/xTk/xTv  [D, S]    inputs pre-transposed on host (feature-major)
  q/k projT    [128, S]  2 local heads stacked on partitions (h0: 0-63)
  scores^T     [128k, q] per 128-wide key chunk; softmax mask depends only
                         on the key position -> per-partition ACT bias
  v_aug        [Sk, 130] per-head [Wv_h | ones] columns; attn@v output row
                         64 of each head block is the softmax denominator
  out          [512, S]  transposed partial output (host transposes back)

The kernel specializes on ceil(max(valid_lens)/128) key chunks: key
positions >= valid_len contribute exactly 0 attention weight (exp of a
large negative bias underflows to 0), so chunks beyond that bound are
skipped entirely.  This is derived from the runtime inputs, so the
kernel stays correct for any valid_lens.
"""

import math
import os
import sys

import numpy as np

for _p in ("/opt/trn_rl_repo",):
    if os.path.isdir(_p) and _p not in sys.path:
        sys.path.insert(0, _p)

import ml_dtypes

D_MODEL = 512
NUM_HEADS = 8
HEAD_DIM = 64
N_CORES = 8
LOCAL_F = 128          # features per core = 2 heads * 64
VAUG = 2 * HEAD_DIM + 2  # 130: [v_h0 (64) | ones | v_h1 (64) | ones]
MASK_NEG = -30000.0

# "bfloat16" or "float32r" (fp32 storage, full-rate matmul w/ reduced mult
# precision) or "float32" (exact, 4x slower matmuls)
DT_NAME = os.environ.get("ATTN_KERNEL_DT", "bfloat16")
TRACE = False

last_results = None  # BassKernelResults of the most recent run (for test.py)

_PROG_CACHE = {}


def _np_dt(name):
    return ml_dtypes.bfloat16 if name == "bfloat16" else np.float32


def _build(nch: int, seq: int, dt_name: str, qk_bias: bool, v_bias: bool):
    from contextlib import ExitStack

    import concourse.bass as bass  # noqa: F401
    import concourse.mybir as mybir
    import concourse.tile as tile
    from concourse import bacc

    DT = getattr(mybir.dt, dt_name)
    F32 = mybir.dt.float32
    EXP = mybir.ActivationFunctionType.Exp
    sk = nch * 128
    n_qt = seq // 512
    n_tp = seq // 1024
    assert seq % 1024 == 0

    nc = bacc.Bacc("TRN2", target_bir_lowering=False, debug=False,
                   num_devices=N_CORES)

    def din(name, shape, dt=DT):
        return nc.dram_tensor(name, shape, dt, kind="ExternalInput").ap()

    xTq = din("xTq", [D_MODEL, seq])
    xTk = din("xTk", [D_MODEL, sk])
    xTv = din("xTv", [D_MODEL, sk])
    wqT = din("wqT", [D_MODEL, LOCAL_F])
    wkT = din("wkT", [D_MODEL, LOCAL_F])
    wvT = din("wvT", [D_MODEL, VAUG])
    woT0 = din("woT0", [HEAD_DIM, D_MODEL])
    woT1 = din("woT1", [HEAD_DIM, D_MODEL])
    bq_d = din("bq", [LOCAL_F, 1], F32)
    bk_d = din("bk", [LOCAL_F, 1], F32)
    bv_d = din("bv", [128, VAUG], F32)
    maskb_d = din("maskb", [128, nch], F32)
    out_d = nc.dram_tensor("out", [D_MODEL, seq], F32,
                           kind="ExternalOutput").ap()
    recip_d = nc.dram_tensor("recip_bounce", [2, seq], F32).ap()

    with tile.TileContext(nc) as tc, ExitStack() as ctx:
        const = ctx.enter_context(tc.tile_pool(name="const", bufs=1))

        # ---- stage inputs into SBUF ----
        xq_sb = const.tile([128, 4, seq], DT, tag="xq")
        nc.sync.dma_start(out=xq_sb, in_=xTq.rearrange("(c p) s -> p c s", p=128))
        xk_sb = const.tile([128, 4, sk], DT, tag="xk")
        nc.sync.dma_start(out=xk_sb, in_=xTk.rearrange("(c p) s -> p c s", p=128))
        xv_sb = const.tile([128, 4, sk], DT, tag="xv")
        nc.sync.dma_start(out=xv_sb, in_=xTv.rearrange("(c p) s -> p c s", p=128))

        wq_sb = const.tile([128, 4, LOCAL_F], DT, tag="wq")
        nc.sync.dma_start(out=wq_sb, in_=wqT.rearrange("(c p) f -> p c f", p=128))
        wk_sb = const.tile([128, 4, LOCAL_F], DT, tag="wk")
        nc.sync.dma_start(out=wk_sb, in_=wkT.rearrange("(c p) f -> p c f", p=128))
        wv_sb = const.tile([128, 4, VAUG], DT, tag="wv")
        nc.sync.dma_start(out=wv_sb, in_=wvT.rearrange("(c p) f -> p c f", p=128))
        wo0_sb = const.tile([HEAD_DIM, D_MODEL], DT, tag="wo0")
        nc.sync.dma_start(out=wo0_sb, in_=woT0)
        wo1_sb = const.tile([HEAD_DIM, D_MODEL], DT, tag="wo1")
        nc.sync.dma_start(out=wo1_sb, in_=woT1)

        bq_sb = const.tile([LOCAL_F, 1], F32, tag="bq")
        nc.sync.dma_start(out=bq_sb, in_=bq_d)
        bk_sb = const.tile([LOCAL_F, 1], F32, tag="bk")
        nc.sync.dma_start(out=bk_sb, in_=bk_d)
        bv_sb = const.tile([128, VAUG], F32, tag="bv")
        nc.sync.dma_start(out=bv_sb, in_=bv_d)
        mb_sb = const.tile([128, nch], F32, tag="mb")
        nc.sync.dma_start(out=mb_sb, in_=maskb_d)

        # ---- projections ----
        qT = const.tile([LOCAL_F, seq], DT, tag="qT")
        kT = const.tile([LOCAL_F, sk], DT, tag="kT")
        vaug = const.tile([128, nch, VAUG], DT, tag="vaug")

        with tc.tile_pool(name="ps_p", bufs=3, space="PSUM") as psp:
            for dst, w_sb, x_sb, b_sb, width in (
                (qT, wq_sb, xq_sb, bq_sb, seq),
                (kT, wk_sb, xk_sb, bk_sb, sk),
            ):
                for j0 in range(0, width, 512):
                    w = min(512, width - j0)
                    ps = psp.tile([128, 512], F32, tag="pp")
                    for dc in range(4):
                        nc.tensor.matmul(
                            ps[:, :w],
                            lhsT=w_sb[:, dc, :],
                            rhs=x_sb[:, dc, j0:j0 + w],
                            start=(dc == 0), stop=(dc == 3),
                        )
                    nc.vector.tensor_copy(out=dst[:, j0:j0 + w], in_=ps[:, :w])
                    if qk_bias:
                        # separate op: TensorScalarPtr has 1 sync-wait slot
                        nc.vector.tensor_scalar_add(
                            out=dst[:, j0:j0 + w], in0=dst[:, j0:j0 + w],
                            scalar1=b_sb)
            nc.vector.memset(vaug[:, :, 64:65], 1.0)
            nc.vector.memset(vaug[:, :, 129:130], 1.0)
            for c in range(nch):
                ps = psp.tile([128, VAUG], F32, tag="ppv")
                for dc in range(4):
                    nc.tensor.matmul(
                        ps,
                        lhsT=xv_sb[:, dc, c * 128:(c + 1) * 128],
                        rhs=wv_sb[:, dc, :],
                        start=(dc == 0), stop=(dc == 3),
                    )
                nc.vector.tensor_copy(out=vaug[:, c, 0:64], in_=ps[:, 0:64])
                nc.vector.tensor_copy(out=vaug[:, c, 65:129], in_=ps[:, 65:129])
                if v_bias:
                    nc.vector.tensor_add(
                        out=vaug[:, c, 0:64], in0=vaug[:, c, 0:64],
                        in1=bv_sb[:, 0:64])
                    nc.vector.tensor_add(
                        out=vaug[:, c, 65:129], in0=vaug[:, c, 65:129],
                        in1=bv_sb[:, 65:129])

        # ---- attention ----
        stage = const.tile([65, 2, seq], F32, tag="stage")
        with (
            tc.tile_pool(name="ps_s", bufs=2, space="PSUM") as pss,
            tc.tile_pool(name="ps_o", bufs=4, space="PSUM") as pso,
            tc.tile_pool(name="expp", bufs=4) as expp,
        ):
            for tp in range(n_tp):
                q0 = tp * 1024
                oT = {}
                for h in range(2):
                    for t in range(2):
                        oT[h, t] = pso.tile([65, 512], F32, tag="oT", name=f"oT{h}{t}")
                for c in range(nch):
                    scs = []
                    for h in range(2):
                        sc = pss.tile([128, 1024], F32, tag="sc")
                        for t in range(2):
                            nc.tensor.matmul(
                                sc[:, t * 512:(t + 1) * 512],
                                lhsT=kT[h * 64:(h + 1) * 64,
                                        c * 128:(c + 1) * 128],
                                rhs=qT[h * 64:(h + 1) * 64,
                                       q0 + t * 512:q0 + (t + 1) * 512],
                                start=True, stop=True,
                            )
                        scs.append(sc)
                    exs = []
                    for h in range(2):
                        ex = expp.tile([128, 1024], DT, tag="ex")
                        nc.scalar.activation(
                            out=ex, in_=scs[h], func=EXP,
                            bias=mb_sb[:, c:c + 1],
                            scale=1.0 / math.sqrt(HEAD_DIM),
                        )
                        exs.append(ex)
                    for h in range(2):
                        for t in range(2):
                            nc.tensor.matmul(
                                oT[h, t],
                                lhsT=vaug[:, c, h * 65:(h + 1) * 65],
                                rhs=exs[h][:, t * 512:(t + 1) * 512],
                                start=(c == 0), stop=(c == nch - 1),
                            )
                for h in range(2):
                    for t in range(2):
                        nc.vector.tensor_copy(
                            out=stage[:, h, q0 + t * 512:q0 + (t + 1) * 512],
                            in_=oT[h, t])

        # ---- normalize ----
        # reciprocal of the denominators, bounced through DRAM to get a
        # partition-broadcast access pattern (SBUF APs need nonzero
        # partition step; DRAM APs don't)
        nc.vector.reciprocal(out=stage[64:65, :, :], in_=stage[64:65, :, :])
        nc.sync.dma_start(out=recip_d, in_=stage[64:65, :, :])
        cns = []
        for h in range(2):
            rb = const.tile([64, seq], F32, tag=f"rb{h}")
            nc.sync.dma_start(
                out=rb, in_=recip_d[h:h + 1, :].to_broadcast([64, seq]))
            cn = const.tile([64, seq], DT, tag=f"cn{h}")
            nc.vector.tensor_mul(out=cn, in0=stage[0:64, h, :], in1=rb)
            cns.append(cn)

        # ---- output projection (transposed partial) ----
        with (
            tc.tile_pool(name="ps_f", bufs=2, space="PSUM") as psf,
            tc.tile_pool(name="outp", bufs=2) as outp,
        ):
            for odc in range(4):
                ob = outp.tile([128, seq], F32, tag="ob")
                for st in range(n_qt):
                    fp = psf.tile([128, 512], F32, tag="fp")
                    nc.tensor.matmul(
                        fp, lhsT=wo0_sb[:, odc * 128:(odc + 1) * 128],
                        rhs=cns[0][:, st * 512:(st + 1) * 512],
                        start=True, stop=False)
                    nc.tensor.matmul(
                        fp, lhsT=wo1_sb[:, odc * 128:(odc + 1) * 128],
                        rhs=cns[1][:, st * 512:(st + 1) * 512],
                        start=False, stop=True)
                    nc.vector.tensor_copy(
                        out=ob[:, st * 512:(st + 1) * 512], in_=fp)
                nc.sync.dma_start(
                    out=out_d[odc * 128:(odc + 1) * 128, :], in_=ob)

    nc.compile()
    return nc


def kernel(queries, keys, values, valid_lens, Wq, bq, Wk, bk, Wv, bv, Wo, bo):
    global last_results
    queries = np.asarray(queries, dtype=np.float32)
    keys = np.asarray(keys, dtype=np.float32)
    values = np.asarray(values, dtype=np.float32)
    valid_lens = np.asarray(valid_lens).astype(np.int64)
    Wq = np.asarray(Wq, dtype=np.float32)
    Wk = np.asarray(Wk, dtype=np.float32)
    Wv = np.asarray(Wv, dtype=np.float32)
    Wo = np.asarray(Wo, dtype=np.float32)
    bq = np.asarray(bq, dtype=np.float32)
    bk = np.asarray(bk, dtype=np.float32)
    bv = np.asarray(bv, dtype=np.float32)
    bo = np.asarray(bo, dtype=np.float32)

    B, S, D = queries.shape
    assert (B, D) == (2, D_MODEL) and S % 1024 == 0

    Lmax = int(min(max(int(valid_lens.max()), 1), S))
    nch = (Lmax + 127) // 128
    sk = nch * 128

    npdt = _np_dt(DT_NAME)
    qk_bias = bool(np.any(bq) or np.any(bk))
    v_bias = bool(np.any(bv))
    key = (nch, S, DT_NAME, qk_bias, v_bias)
    if key not in _PROG_CACHE:
        _PROG_CACHE[key] = _build(nch, S, DT_NAME, qk_bias, v_bias)
    nc = _PROG_CACHE[key]

    in_maps = []
    for core in range(N_CORES):
        b, hp = divmod(core, 4)
        L = int(valid_lens[b])
        fs = hp * LOCAL_F
        wvT_aug = np.zeros((D, VAUG), np.float32)
        wvT_aug[:, 0:64] = Wv[fs:fs + 64, :].T
        wvT_aug[:, 65:129] = Wv[fs + 64:fs + 128, :].T
        bv_aug = np.zeros((VAUG,), np.float32)
        bv_aug[0:64] = bv[fs:fs + 64]
        bv_aug[64] = 1.0
        bv_aug[65:129] = bv[fs + 64:fs + 128]
        bv_aug[129] = 1.0
        if L == 0:
            mask = np.zeros((sk,), np.float32)  # result discarded on host
        else:
            mask = np.where(np.arange(sk) < L, 0.0, MASK_NEG).astype(np.float32)
        in_maps.append({
            "xTq": np.ascontiguousarray(queries[b].T).astype(npdt),
            "xTk": np.ascontiguousarray(keys[b, :sk].T).astype(npdt),
            "xTv": np.ascontiguousarray(values[b, :sk].T).astype(npdt),
            "wqT": np.ascontiguousarray(Wq[fs:fs + 128, :].T).astype(npdt),
            "wkT": np.ascontiguousarray(Wk[fs:fs + 128, :].T).astype(npdt),
            "wvT": wvT_aug.astype(npdt),
            "woT0": np.ascontiguousarray(Wo[:, fs:fs + 64].T).astype(npdt),
            "woT1": np.ascontiguousarray(Wo[:, fs + 64:fs + 128].T).astype(npdt),
            "bq": bq[fs:fs + 128].reshape(LOCAL_F, 1).copy(),
            "bk": bk[fs:fs + 128].reshape(LOCAL_F, 1).copy(),
            "bv": np.ascontiguousarray(
                np.broadcast_to(bv_aug, (128, VAUG))).astype(np.float32),
            "maskb": np.ascontiguousarray(
                mask.reshape(nch, 128).T).astype(np.float32),
        })

    from concourse.bass_utils import run_bass_kernel_spmd
    res = run_bass_kernel_spmd(nc, in_maps, list(range(N_CORES)), trace=TRACE)
    last_results = res
    outs = [r["out"] for r in res.results]

    final = np.empty((B, S, D), np.float32)
    for b in range(B):
        acc = outs[4 * b] + outs[4 * b + 1] + outs[4 * b + 2] + outs[4 * b + 3]
        final[b] = acc.T + bo
        if int(valid_lens[b]) == 0:
            # uniform attention over all S positions (reference semantics
            # when every key is masked: softmax of a constant row)
            row = (values[b].mean(0) @ Wv.T + bv) @ Wo.T + bo
            final[b] = np.broadcast_to(row, (S, D))
    return final


# revision 18
# speedup vs baseline: 1.0363x; 1.0363x over previous
"""Trainium2 Bass kernel: masked multi-head attention (B=2, S=2048, D=512, H=8).

Sharding: batch x head-pair across 8 cores (core = b*4 + head_pair).
Each core computes, for its batch b and its 2 heads:
    q/k/v projections -> scores^T -> exp (mask folded in as per-partition
    bias on the ScalarE) -> attn@v with a ones-column appended to V (gives
    the softmax denominator for free) -> normalize -> partial out-proj.
The 4 per-batch partials are summed on the host (the "all-reduce"), then
bias bo is added.

Device layouts (per core):
  xTq/xTk/xTv  [D, S]    inputs pre-transposed on host (feature-major)
  q/k projT    [128, S]  2 local heads stacked on partitions (h0: 0-63)
  scores^T     [128k, q] per 128-wide key chunk; softmax mask depends only
                         on the key position -> per-partition ACT bias
  v_aug        [Sk, 130] per-head [Wv_h | ones] columns; attn@v output row
                         64 of each head block is the softmax denominator
  out          [512, S]  transposed partial output (host transposes back)

The kernel specializes on ceil(max(valid_lens)/128) key chunks: key
positions >= valid_len contribute exactly 0 attention weight (exp of a
large negative bias underflows to 0), so chunks beyond that bound are
skipped entirely.  This is derived from the runtime inputs, so the
kernel stays correct for any valid_lens.
"""

import math
import os
import sys

import numpy as np

for _p in ("/opt/trn_rl_repo",):
    if os.path.isdir(_p) and _p not in sys.path:
        sys.path.insert(0, _p)

import ml_dtypes

D_MODEL = 512
NUM_HEADS = 8
HEAD_DIM = 64
N_CORES = 8
LOCAL_F = 128          # features per core = 2 heads * 64
VAUG = 2 * HEAD_DIM + 2  # 130: [v_h0 (64) | ones | v_h1 (64) | ones]
MASK_NEG = -30000.0

# "bfloat16" or "float32r" (fp32 storage, full-rate matmul w/ reduced mult
# precision) or "float32" (exact, 4x slower matmuls)
DT_NAME = os.environ.get("ATTN_KERNEL_DT", "bfloat16")
TRACE = False

last_results = None  # BassKernelResults of the most recent run (for test.py)

_PROG_CACHE = {}


def _np_dt(name):
    return ml_dtypes.bfloat16 if name == "bfloat16" else np.float32


def _build(nch: int, seq: int, dt_name: str, qk_bias: bool, v_bias: bool):
    from contextlib import ExitStack

    import concourse.bass as bass  # noqa: F401
    import concourse.mybir as mybir
    import concourse.tile as tile
    from concourse import bacc

    DT = getattr(mybir.dt, dt_name)
    F32 = mybir.dt.float32
    EXP = mybir.ActivationFunctionType.Exp
    sk = nch * 128
    n_qt = seq // 512
    n_tp = seq // 1024
    assert seq % 1024 == 0

    nc = bacc.Bacc("TRN2", target_bir_lowering=False, debug=False,
                   num_devices=N_CORES)

    def din(name, shape, dt=DT):
        return nc.dram_tensor(name, shape, dt, kind="ExternalInput").ap()

    xTq = din("xTq", [D_MODEL, seq])
    xTk = din("xTk", [D_MODEL, sk])
    xTv = din("xTv", [D_MODEL, sk])
    # [wqT | wkT | wvT_aug] column blocks
    WQKV = 2 * LOCAL_F + VAUG
    wqkv = din("wqkv", [D_MODEL, WQKV])
    woT0 = din("woT0", [HEAD_DIM, D_MODEL])
    woT1 = din("woT1", [HEAD_DIM, D_MODEL])
    # f32 smalls: [bq | bk | bv_aug(VAUG) | maskb(nch)]
    NSM = 2 + VAUG + nch
    smalls_d = din("smalls", [128, NSM], F32)
    out_d = nc.dram_tensor("out", [D_MODEL, seq], F32,
                           kind="ExternalOutput").ap()
    recip_d = nc.dram_tensor("recip_bounce", [2, seq], F32).ap()

    with tile.TileContext(nc) as tc, ExitStack() as ctx:
        const = ctx.enter_context(tc.tile_pool(name="const", bufs=1))

        # ---- stage inputs into SBUF ----
        # smalls + weights on the SWDGE queue (parallel with the big input
        # loads on the sync HWDGE queue); inputs column-split so compute
        # can start before staging completes
        sm_sb = const.tile([128, NSM], F32, tag="sm")
        nc.gpsimd.dma_start(out=sm_sb, in_=smalls_d)
        wo0_sb = const.tile([HEAD_DIM, D_MODEL], DT, tag="wo0")
        nc.gpsimd.dma_start(out=wo0_sb, in_=woT0)
        wo1_sb = const.tile([HEAD_DIM, D_MODEL], DT, tag="wo1")
        nc.gpsimd.dma_start(out=wo1_sb, in_=woT1)
        wqkv_sb = const.tile([128, 4, WQKV], DT, tag="wqkv")
        nc.gpsimd.dma_start(
            out=wqkv_sb, in_=wqkv.rearrange("(c p) f -> p c f", p=128))

        xk_sb = const.tile([128, 4, sk], DT, tag="xk")
        nc.sync.dma_start(out=xk_sb, in_=xTk.rearrange("(c p) s -> p c s", p=128))
        xq_sb = const.tile([128, 4, seq], DT, tag="xq")
        xq_r = xTq.rearrange("(c p) s -> p c s", p=128)
        for j0 in range(0, seq, 512):
            nc.sync.dma_start(out=xq_sb[:, :, j0:j0 + 512],
                              in_=xq_r[:, :, j0:j0 + 512])
        xv_sb = const.tile([128, 4, sk], DT, tag="xv")
        nc.sync.dma_start(out=xv_sb, in_=xTv.rearrange("(c p) s -> p c s", p=128))

        bq_sb = sm_sb[:, 0:1]
        bk_sb = sm_sb[:, 1:2]
        bv_sb = sm_sb[:, 2:2 + VAUG]
        mb_sb = sm_sb[:, 2 + VAUG:2 + VAUG + nch]
        wq_of, wk_of, wv_of = 0, LOCAL_F, 2 * LOCAL_F

        # ---- projections ----
        qT = const.tile([LOCAL_F, seq], DT, tag="qT")
        kT = const.tile([LOCAL_F, sk], DT, tag="kT")
        vaug = const.tile([128, nch, VAUG], DT, tag="vaug")

        with tc.tile_pool(name="ps_p", bufs=3, space="PSUM") as psp:
            for dst, w_of, x_sb, b_sb, width in (
                (kT, wk_of, xk_sb, bk_sb, sk),
                (qT, wq_of, xq_sb, bq_sb, seq),
            ):
                for j0 in range(0, width, 512):
                    w = min(512, width - j0)
                    ps = psp.tile([128, 512], F32, tag="pp")
                    for dc in range(4):
                        nc.tensor.matmul(
                            ps[:, :w],
                            lhsT=wqkv_sb[:, dc, w_of:w_of + LOCAL_F],
                            rhs=x_sb[:, dc, j0:j0 + w],
                            start=(dc == 0), stop=(dc == 3),
                        )
                    nc.vector.tensor_copy(out=dst[:, j0:j0 + w], in_=ps[:, :w])
                    if qk_bias:
                        # separate op: TensorScalarPtr has 1 sync-wait slot
                        nc.vector.tensor_scalar_add(
                            out=dst[:, j0:j0 + w], in0=dst[:, j0:j0 + w],
                            scalar1=b_sb)
            nc.vector.memset(vaug[:, :, 64:65], 1.0)
            nc.vector.memset(vaug[:, :, 129:130], 1.0)
            for c in range(nch):
                ps = psp.tile([128, VAUG], F32, tag="ppv")
                for dc in range(4):
                    nc.tensor.matmul(
                        ps,
                        lhsT=xv_sb[:, dc, c * 128:(c + 1) * 128],
                        rhs=wqkv_sb[:, dc, wv_of:wv_of + VAUG],
                        start=(dc == 0), stop=(dc == 3),
                    )
                nc.vector.tensor_copy(out=vaug[:, c, 0:64], in_=ps[:, 0:64])
                nc.vector.tensor_copy(out=vaug[:, c, 65:129], in_=ps[:, 65:129])
                if v_bias:
                    nc.vector.tensor_add(
                        out=vaug[:, c, 0:64], in0=vaug[:, c, 0:64],
                        in1=bv_sb[:, 0:64])
                    nc.vector.tensor_add(
                        out=vaug[:, c, 65:129], in0=vaug[:, c, 65:129],
                        in1=bv_sb[:, 65:129])

        # ---- attention ----
        stage = const.tile([65, 2, seq], F32, tag="stage")
        with (
            tc.tile_pool(name="ps_s", bufs=2, space="PSUM") as pss,
            tc.tile_pool(name="ps_o", bufs=4, space="PSUM") as pso,
            tc.tile_pool(name="expp", bufs=4) as expp,
        ):
            for tp in range(n_tp):
                q0 = tp * 1024
                oT = {}
                for h in range(2):
                    for t in range(2):
                        oT[h, t] = pso.tile([65, 512], F32, tag="oT", name=f"oT{h}{t}")
                for c in range(nch):
                    scs = []
                    for h in range(2):
                        sc = pss.tile([128, 1024], F32, tag="sc")
                        for t in range(2):
                            nc.tensor.matmul(
                                sc[:, t * 512:(t + 1) * 512],
                                lhsT=kT[h * 64:(h + 1) * 64,
                                        c * 128:(c + 1) * 128],
                                rhs=qT[h * 64:(h + 1) * 64,
                                       q0 + t * 512:q0 + (t + 1) * 512],
                                start=True, stop=True,
                            )
                        scs.append(sc)
                    exs = []
                    for h in range(2):
                        ex = expp.tile([128, 1024], DT, tag="ex")
                        nc.scalar.activation(
                            out=ex, in_=scs[h], func=EXP,
                            bias=mb_sb[:, c:c + 1],
                            scale=1.0 / math.sqrt(HEAD_DIM),
                        )
                        exs.append(ex)
                    for h in range(2):
                        for t in range(2):
                            nc.tensor.matmul(
                                oT[h, t],
                                lhsT=vaug[:, c, h * 65:(h + 1) * 65],
                                rhs=exs[h][:, t * 512:(t + 1) * 512],
                                start=(c == 0), stop=(c == nch - 1),
                            )
                for h in range(2):
                    for t in range(2):
                        nc.vector.tensor_copy(
                            out=stage[:, h, q0 + t * 512:q0 + (t + 1) * 512],
                            in_=oT[h, t])

        # ---- normalize ----
        # denominators bounced through DRAM to get a partition-broadcast
        # access pattern (SBUF APs need nonzero partition step; DRAM APs
        # don't); reciprocal AFTER the broadcast so all 64 lanes work
        nc.sync.dma_start(out=recip_d, in_=stage[64:65, :, :])
        cns = []
        for h in range(2):
            rb = const.tile([64, seq], F32, tag=f"rb{h}")
            nc.sync.dma_start(
                out=rb, in_=recip_d[h:h + 1, :].to_broadcast([64, seq]))
            nc.vector.reciprocal(out=rb, in_=rb)
            cn = const.tile([64, seq], DT, tag=f"cn{h}")
            nc.vector.tensor_mul(out=cn, in0=stage[0:64, h, :], in1=rb)
            cns.append(cn)

        # ---- output projection (transposed partial) ----
        with (
            tc.tile_pool(name="ps_f", bufs=2, space="PSUM") as psf,
            tc.tile_pool(name="outp", bufs=2) as outp,
        ):
            for odc in range(4):
                ob = outp.tile([128, seq], F32, tag="ob")
                for st in range(n_qt):
                    fp = psf.tile([128, 512], F32, tag="fp")
                    nc.tensor.matmul(
                        fp, lhsT=wo0_sb[:, odc * 128:(odc + 1) * 128],
                        rhs=cns[0][:, st * 512:(st + 1) * 512],
                        start=True, stop=False)
                    nc.tensor.matmul(
                        fp, lhsT=wo1_sb[:, odc * 128:(odc + 1) * 128],
                        rhs=cns[1][:, st * 512:(st + 1) * 512],
                        start=False, stop=True)
                    if st % 2 == 0:
                        nc.vector.tensor_copy(
                            out=ob[:, st * 512:(st + 1) * 512], in_=fp)
                    else:
                        nc.scalar.copy(
                            out=ob[:, st * 512:(st + 1) * 512], in_=fp)
                nc.sync.dma_start(
                    out=out_d[odc * 128:(odc + 1) * 128, :], in_=ob)

    nc.compile()
    return nc


def kernel(queries, keys, values, valid_lens, Wq, bq, Wk, bk, Wv, bv, Wo, bo):
    global last_results
    queries = np.asarray(queries, dtype=np.float32)
    keys = np.asarray(keys, dtype=np.float32)
    values = np.asarray(values, dtype=np.float32)
    valid_lens = np.asarray(valid_lens).astype(np.int64)
    Wq = np.asarray(Wq, dtype=np.float32)
    Wk = np.asarray(Wk, dtype=np.float32)
    Wv = np.asarray(Wv, dtype=np.float32)
    Wo = np.asarray(Wo, dtype=np.float32)
    bq = np.asarray(bq, dtype=np.float32)
    bk = np.asarray(bk, dtype=np.float32)
    bv = np.asarray(bv, dtype=np.float32)
    bo = np.asarray(bo, dtype=np.float32)

    B, S, D = queries.shape
    assert (B, D) == (2, D_MODEL) and S % 1024 == 0

    Lmax = int(min(max(int(valid_lens.max()), 1), S))
    nch = (Lmax + 127) // 128
    sk = nch * 128

    npdt = _np_dt(DT_NAME)
    qk_bias = bool(np.any(bq) or np.any(bk))
    v_bias = bool(np.any(bv))
    key = (nch, S, DT_NAME, qk_bias, v_bias)
    if key not in _PROG_CACHE:
        _PROG_CACHE[key] = _build(nch, S, DT_NAME, qk_bias, v_bias)
    nc = _PROG_CACHE[key]

    in_maps = []
    for core in range(N_CORES):
        b, hp = divmod(core, 4)
        L = int(valid_lens[b])
        fs = hp * LOCAL_F
        wvT_aug = np.zeros((D, VAUG), np.float32)
        wvT_aug[:, 0:64] = Wv[fs:fs + 64, :].T
        wvT_aug[:, 65:129] = Wv[fs + 64:fs + 128, :].T
        bv_aug = np.zeros((VAUG,), np.float32)
        bv_aug[0:64] = bv[fs:fs + 64]
        bv_aug[64] = 1.0
        bv_aug[65:129] = bv[fs + 64:fs + 128]
        bv_aug[129] = 1.0
        if L == 0:
            mask = np.zeros((sk,), np.float32)  # result discarded on host
        else:
            mask = np.where(np.arange(sk) < L, 0.0, MASK_NEG).astype(np.float32)
        wqkv = np.concatenate(
            [Wq[fs:fs + 128, :].T, Wk[fs:fs + 128, :].T, wvT_aug], axis=1)
        smalls = np.empty((128, 2 + VAUG + nch), np.float32)
        smalls[:, 0] = bq[fs:fs + 128]
        smalls[:, 1] = bk[fs:fs + 128]
        smalls[:, 2:2 + VAUG] = bv_aug
        smalls[:, 2 + VAUG:] = mask.reshape(nch, 128).T
        in_maps.append({
            "xTq": np.ascontiguousarray(queries[b].T).astype(npdt),
            "xTk": np.ascontiguousarray(keys[b, :sk].T).astype(npdt),
            "xTv": np.ascontiguousarray(values[b, :sk].T).astype(npdt),
            "wqkv": np.ascontiguousarray(wqkv).astype(npdt),
            "woT0": np.ascontiguousarray(Wo[:, fs:fs + 64].T).astype(npdt),
            "woT1": np.ascontiguousarray(Wo[:, fs + 64:fs + 128].T).astype(npdt),
            "smalls": smalls,
        })

    from concourse.bass_utils import run_bass_kernel_spmd
    res = run_bass_kernel_spmd(nc, in_maps, list(range(N_CORES)), trace=TRACE)
    last_results = res
    outs = [r["out"] for r in res.results]

    final = np.empty((B, S, D), np.float32)
    for b in range(B):
        acc = outs[4 * b] + outs[4 * b + 1] + outs[4 * b + 2] + outs[4 * b + 3]
        final[b] = acc.T + bo
        if int(valid_lens[b]) == 0:
            # uniform attention over all S positions (reference semantics
            # when every key is masked: softmax of a constant row)
            row = (values[b].mean(0) @ Wv.T + bv) @ Wo.T + bo
            final[b] = np.broadcast_to(row, (S, D))
    return final


# revision 19
# speedup vs baseline: 1.2338x; 1.1906x over previous
"""Trainium2 Bass kernel: masked multi-head attention (B=2, S=2048, D=512, H=8).

Sharding: batch x head-pair across 8 cores (core = b*4 + head_pair).
Each core computes, for its batch b and its 2 heads:
    q/k/v projections -> scores^T -> exp (mask folded in as per-partition
    bias on the ScalarE) -> attn@v with a ones-column appended to V (gives
    the softmax denominator for free) -> normalize -> partial out-proj.
The 4 per-batch partials are summed on the host (the "all-reduce"), then
bias bo is added.

Device layouts (per core):
  xTq/xTk/xTv  [D, S]    inputs pre-transposed on host (feature-major)
  q/k projT    [128, S]  2 local heads stacked on partitions (h0: 0-63)
  scores^T     [128k, q] per 128-wide key chunk; softmax mask depends only
                         on the key position -> per-partition ACT bias
  v_aug        [Sk, 130] per-head [Wv_h | ones] columns; attn@v output row
                         64 of each head block is the softmax denominator
  out          [512, S]  transposed partial output (host transposes back)

The kernel specializes on ceil(max(valid_lens)/128) key chunks: key
positions >= valid_len contribute exactly 0 attention weight (exp of a
large negative bias underflows to 0), so chunks beyond that bound are
skipped entirely.  This is derived from the runtime inputs, so the
kernel stays correct for any valid_lens.
"""

import math
import os
import sys

import numpy as np

for _p in ("/opt/trn_rl_repo",):
    if os.path.isdir(_p) and _p not in sys.path:
        sys.path.insert(0, _p)

import ml_dtypes

D_MODEL = 512
NUM_HEADS = 8
HEAD_DIM = 64
N_CORES = 8
LOCAL_F = 128          # features per core = 2 heads * 64
VAUG = 2 * HEAD_DIM + 2  # 130: [v_h0 (64) | ones | v_h1 (64) | ones]
MASK_NEG = -30000.0

# "bfloat16" or "float32r" (fp32 storage, full-rate matmul w/ reduced mult
# precision) or "float32" (exact, 4x slower matmuls)
DT_NAME = os.environ.get("ATTN_KERNEL_DT", "bfloat16")
TRACE = False

last_results = None  # BassKernelResults of the most recent run (for test.py)

_PROG_CACHE = {}


def _np_dt(name):
    return ml_dtypes.bfloat16 if name == "bfloat16" else np.float32


def _build(nch: int, seq: int, dt_name: str, qk_bias: bool, v_bias: bool):
    from contextlib import ExitStack

    import concourse.bass as bass  # noqa: F401
    import concourse.mybir as mybir
    import concourse.tile as tile
    from concourse import bacc

    DT = getattr(mybir.dt, dt_name)
    F32 = mybir.dt.float32
    EXP = mybir.ActivationFunctionType.Exp
    sk = nch * 128
    n_qt = seq // 512
    n_tp = seq // 1024
    assert seq % 1024 == 0

    nc = bacc.Bacc("TRN2", target_bir_lowering=False, debug=False,
                   num_devices=N_CORES)

    def din(name, shape, dt=DT):
        return nc.dram_tensor(name, shape, dt, kind="ExternalInput").ap()

    xTq = din("xTq", [D_MODEL, seq])
    xTk = din("xTk", [D_MODEL, sk])
    xTv = din("xTv", [D_MODEL, sk])
    # [wqT | wkT | wvT_aug] column blocks
    WQKV = 2 * LOCAL_F + VAUG
    wqkv = din("wqkv", [D_MODEL, WQKV])
    woT0 = din("woT0", [HEAD_DIM, D_MODEL])
    woT1 = din("woT1", [HEAD_DIM, D_MODEL])
    # f32 smalls: [bq | bk | bv_aug(VAUG) | maskb(nch)]
    NSM = 2 + VAUG + nch
    smalls_d = din("smalls", [128, NSM], F32)
    out_d = nc.dram_tensor("out", [D_MODEL, seq], F32,
                           kind="ExternalOutput").ap()
    recip_d = nc.dram_tensor("recip_bounce", [2, seq], F32).ap()

    with tile.TileContext(nc) as tc, ExitStack() as ctx:
        const = ctx.enter_context(tc.tile_pool(name="const", bufs=1))

        # ---- stage inputs into SBUF ----
        # smalls + weights on the SWDGE queue (parallel with the big input
        # loads on the sync HWDGE queue); inputs column-split so compute
        # can start before staging completes
        sm_sb = const.tile([128, NSM], F32, tag="sm")
        nc.gpsimd.dma_start(out=sm_sb, in_=smalls_d)
        wo0_sb = const.tile([HEAD_DIM, D_MODEL], DT, tag="wo0")
        nc.gpsimd.dma_start(out=wo0_sb, in_=woT0)
        wo1_sb = const.tile([HEAD_DIM, D_MODEL], DT, tag="wo1")
        nc.gpsimd.dma_start(out=wo1_sb, in_=woT1)
        wqkv_sb = const.tile([128, 4, WQKV], DT, tag="wqkv")
        nc.gpsimd.dma_start(
            out=wqkv_sb, in_=wqkv.rearrange("(c p) f -> p c f", p=128))

        xk_sb = const.tile([128, 4, sk], DT, tag="xk")
        nc.sync.dma_start(out=xk_sb, in_=xTk.rearrange("(c p) s -> p c s", p=128))
        xq_sb = const.tile([128, 4, seq], DT, tag="xq")
        xq_r = xTq.rearrange("(c p) s -> p c s", p=128)
        for j0 in range(0, seq, 512):
            nc.sync.dma_start(out=xq_sb[:, :, j0:j0 + 512],
                              in_=xq_r[:, :, j0:j0 + 512])
        xv_sb = const.tile([128, 4, sk], DT, tag="xv")
        nc.sync.dma_start(out=xv_sb, in_=xTv.rearrange("(c p) s -> p c s", p=128))

        bq_sb = sm_sb[:, 0:1]
        bk_sb = sm_sb[:, 1:2]
        bv_sb = sm_sb[:, 2:2 + VAUG]
        mb_sb = sm_sb[:, 2 + VAUG:2 + VAUG + nch]
        wq_of, wk_of, wv_of = 0, LOCAL_F, 2 * LOCAL_F

        # ---- projections ----
        qT = const.tile([LOCAL_F, seq], DT, tag="qT")
        kT = const.tile([LOCAL_F, sk], DT, tag="kT")
        vaug = const.tile([128, nch, VAUG], DT, tag="vaug")

        with tc.tile_pool(name="ps_p", bufs=3, space="PSUM") as psp:
            for dst, w_of, x_sb, b_sb, width in (
                (kT, wk_of, xk_sb, bk_sb, sk),
                (qT, wq_of, xq_sb, bq_sb, seq),
            ):
                for j0 in range(0, width, 512):
                    w = min(512, width - j0)
                    ps = psp.tile([128, 512], F32, tag="pp")
                    for dc in range(4):
                        nc.tensor.matmul(
                            ps[:, :w],
                            lhsT=wqkv_sb[:, dc, w_of:w_of + LOCAL_F],
                            rhs=x_sb[:, dc, j0:j0 + w],
                            start=(dc == 0), stop=(dc == 3),
                        )
                    nc.vector.tensor_copy(out=dst[:, j0:j0 + w], in_=ps[:, :w])
                    if qk_bias:
                        # separate op: TensorScalarPtr has 1 sync-wait slot
                        nc.vector.tensor_scalar_add(
                            out=dst[:, j0:j0 + w], in0=dst[:, j0:j0 + w],
                            scalar1=b_sb)
            nc.vector.memset(vaug[:, :, 64:65], 1.0)
            nc.vector.memset(vaug[:, :, 129:130], 1.0)
            for c in range(nch):
                ps = psp.tile([128, VAUG], F32, tag="ppv")
                for dc in range(4):
                    nc.tensor.matmul(
                        ps,
                        lhsT=xv_sb[:, dc, c * 128:(c + 1) * 128],
                        rhs=wqkv_sb[:, dc, wv_of:wv_of + VAUG],
                        start=(dc == 0), stop=(dc == 3),
                    )
                nc.vector.tensor_copy(out=vaug[:, c, 0:64], in_=ps[:, 0:64])
                nc.vector.tensor_copy(out=vaug[:, c, 65:129], in_=ps[:, 65:129])
                if v_bias:
                    nc.vector.tensor_add(
                        out=vaug[:, c, 0:64], in0=vaug[:, c, 0:64],
                        in1=bv_sb[:, 0:64])
                    nc.vector.tensor_add(
                        out=vaug[:, c, 65:129], in0=vaug[:, c, 65:129],
                        in1=bv_sb[:, 65:129])

        # ---- attention ----
        stage = const.tile([65, 2, seq], F32, tag="stage")
        with (
            tc.tile_pool(name="ps_s", bufs=2, space="PSUM") as pss,
            tc.tile_pool(name="ps_o", bufs=4, space="PSUM") as pso,
            tc.tile_pool(name="expp", bufs=4) as expp,
        ):
            for tp in range(n_tp):
                q0 = tp * 1024
                oT = {}
                for h in range(2):
                    for t in range(2):
                        oT[h, t] = pso.tile([65, 512], F32, tag="oT", name=f"oT{h}{t}")
                for c in range(nch):
                    scs = []
                    for h in range(2):
                        sc = pss.tile([128, 1024], F32, tag="sc")
                        for t in range(2):
                            nc.tensor.matmul(
                                sc[:, t * 512:(t + 1) * 512],
                                lhsT=kT[h * 64:(h + 1) * 64,
                                        c * 128:(c + 1) * 128],
                                rhs=qT[h * 64:(h + 1) * 64,
                                       q0 + t * 512:q0 + (t + 1) * 512],
                                start=True, stop=True,
                            )
                        scs.append(sc)
                    exs = []
                    for h in range(2):
                        ex = expp.tile([128, 1024], DT, tag="ex")
                        nc.scalar.activation(
                            out=ex, in_=scs[h], func=EXP,
                            bias=mb_sb[:, c:c + 1],
                            scale=1.0 / math.sqrt(HEAD_DIM),
                        )
                        exs.append(ex)
                    for h in range(2):
                        for t in range(2):
                            nc.tensor.matmul(
                                oT[h, t],
                                lhsT=vaug[:, c, h * 65:(h + 1) * 65],
                                rhs=exs[h][:, t * 512:(t + 1) * 512],
                                start=(c == 0), stop=(c == nch - 1),
                            )
                for h in range(2):
                    for t in range(2):
                        nc.vector.tensor_copy(
                            out=stage[:, h, q0 + t * 512:q0 + (t + 1) * 512],
                            in_=oT[h, t])

        # ---- normalize ----
        # denominators bounced through DRAM to get a partition-broadcast
        # access pattern (SBUF APs need nonzero partition step; DRAM APs
        # don't); reciprocal AFTER the broadcast so all 64 lanes work
        nc.sync.dma_start(out=recip_d, in_=stage[64:65, :, :])
        cns = []
        for h in range(2):
            rb = const.tile([64, seq], F32, tag=f"rb{h}")
            nc.sync.dma_start(
                out=rb, in_=recip_d[h:h + 1, :].to_broadcast([64, seq]))
            nc.vector.reciprocal_approx_fast(out=rb, in_=rb)
            cn = const.tile([64, seq], DT, tag=f"cn{h}")
            nc.vector.tensor_mul(out=cn, in0=stage[0:64, h, :], in1=rb)
            cns.append(cn)

        # ---- output projection (transposed partial) ----
        with (
            tc.tile_pool(name="ps_f", bufs=2, space="PSUM") as psf,
            tc.tile_pool(name="outp", bufs=2) as outp,
        ):
            for odc in range(4):
                ob = outp.tile([128, seq], F32, tag="ob")
                for st in range(n_qt):
                    fp = psf.tile([128, 512], F32, tag="fp")
                    nc.tensor.matmul(
                        fp, lhsT=wo0_sb[:, odc * 128:(odc + 1) * 128],
                        rhs=cns[0][:, st * 512:(st + 1) * 512],
                        start=True, stop=False)
                    nc.tensor.matmul(
                        fp, lhsT=wo1_sb[:, odc * 128:(odc + 1) * 128],
                        rhs=cns[1][:, st * 512:(st + 1) * 512],
                        start=False, stop=True)
                    if st % 2 == 0:
                        nc.vector.tensor_copy(
                            out=ob[:, st * 512:(st + 1) * 512], in_=fp)
                    else:
                        nc.scalar.copy(
                            out=ob[:, st * 512:(st + 1) * 512], in_=fp)
                nc.sync.dma_start(
                    out=out_d[odc * 128:(odc + 1) * 128, :], in_=ob)

    nc.compile()
    return nc


def kernel(queries, keys, values, valid_lens, Wq, bq, Wk, bk, Wv, bv, Wo, bo):
    global last_results
    queries = np.asarray(queries, dtype=np.float32)
    keys = np.asarray(keys, dtype=np.float32)
    values = np.asarray(values, dtype=np.float32)
    valid_lens = np.asarray(valid_lens).astype(np.int64)
    Wq = np.asarray(Wq, dtype=np.float32)
    Wk = np.asarray(Wk, dtype=np.float32)
    Wv = np.asarray(Wv, dtype=np.float32)
    Wo = np.asarray(Wo, dtype=np.float32)
    bq = np.asarray(bq, dtype=np.float32)
    bk = np.asarray(bk, dtype=np.float32)
    bv = np.asarray(bv, dtype=np.float32)
    bo = np.asarray(bo, dtype=np.float32)

    B, S, D = queries.shape
    assert (B, D) == (2, D_MODEL) and S % 1024 == 0

    Lmax = int(min(max(int(valid_lens.max()), 1), S))
    nch = (Lmax + 127) // 128
    sk = nch * 128

    npdt = _np_dt(DT_NAME)
    qk_bias = bool(np.any(bq) or np.any(bk))
    v_bias = bool(np.any(bv))
    key = (nch, S, DT_NAME, qk_bias, v_bias)
    if key not in _PROG_CACHE:
        _PROG_CACHE[key] = _build(nch, S, DT_NAME, qk_bias, v_bias)
    nc = _PROG_CACHE[key]

    in_maps = []
    for core in range(N_CORES):
        b, hp = divmod(core, 4)
        L = int(valid_lens[b])
        fs = hp * LOCAL_F
        wvT_aug = np.zeros((D, VAUG), np.float32)
        wvT_aug[:, 0:64] = Wv[fs:fs + 64, :].T
        wvT_aug[:, 65:129] = Wv[fs + 64:fs + 128, :].T
        bv_aug = np.zeros((VAUG,), np.float32)
        bv_aug[0:64] = bv[fs:fs + 64]
        bv_aug[64] = 1.0
        bv_aug[65:129] = bv[fs + 64:fs + 128]
        bv_aug[129] = 1.0
        if L == 0:
            mask = np.zeros((sk,), np.float32)  # result discarded on host
        else:
            mask = np.where(np.arange(sk) < L, 0.0, MASK_NEG).astype(np.float32)
        wqkv = np.concatenate(
            [Wq[fs:fs + 128, :].T, Wk[fs:fs + 128, :].T, wvT_aug], axis=1)
        smalls = np.empty((128, 2 + VAUG + nch), np.float32)
        smalls[:, 0] = bq[fs:fs + 128]
        smalls[:, 1] = bk[fs:fs + 128]
        smalls[:, 2:2 + VAUG] = bv_aug
        smalls[:, 2 + VAUG:] = mask.reshape(nch, 128).T
        in_maps.append({
            "xTq": np.ascontiguousarray(queries[b].T).astype(npdt),
            "xTk": np.ascontiguousarray(keys[b, :sk].T).astype(npdt),
            "xTv": np.ascontiguousarray(values[b, :sk].T).astype(npdt),
            "wqkv": np.ascontiguousarray(wqkv).astype(npdt),
            "woT0": np.ascontiguousarray(Wo[:, fs:fs + 64].T).astype(npdt),
            "woT1": np.ascontiguousarray(Wo[:, fs + 64:fs + 128].T).astype(npdt),
            "smalls": smalls,
        })

    from concourse.bass_utils import run_bass_kernel_spmd
    res = run_bass_kernel_spmd(nc, in_maps, list(range(N_CORES)), trace=TRACE)
    last_results = res
    outs = [r["out"] for r in res.results]

    final = np.empty((B, S, D), np.float32)
    for b in range(B):
        acc = outs[4 * b] + outs[4 * b + 1] + outs[4 * b + 2] + outs[4 * b + 3]
        final[b] = acc.T + bo
        if int(valid_lens[b]) == 0:
            # uniform attention over all S positions (reference semantics
            # when every key is masked: softmax of a constant row)
            row = (values[b].mean(0) @ Wv.T + bv) @ Wo.T + bo
            final[b] = np.broadcast_to(row, (S, D))
    return final


# revision 27
# speedup vs baseline: 1.3425x; 1.0881x over previous
"""Trainium2 Bass kernel: masked multi-head attention (B=2, S=2048, D=512, H=8).

Sharding: batch x head-pair across 8 cores (core = b*4 + head_pair).
Each core computes, for its batch b and its 2 heads:
    q/k/v projections -> scores^T -> exp (mask folded in as per-partition
    bias on the ScalarE) -> attn@v with a ones-column appended to V (gives
    the softmax denominator for free) -> normalize -> partial out-proj.
The 4 per-batch partials are summed on the host (the "all-reduce"), then
bias bo is added.

Device layouts (per core):
  xTq/xTk/xTv  [D, S]    inputs pre-transposed on host (feature-major)
  q/k projT    [128, S]  2 local heads stacked on partitions (h0: 0-63)
  scores^T     [128k, q] per 128-wide key chunk; softmax mask depends only
                         on the key position -> per-partition ACT bias
  v_aug        [Sk, 130] per-head [Wv_h | ones] columns; attn@v output row
                         64 of each head block is the softmax denominator
  out          [512, S]  transposed partial output (host transposes back)

The kernel specializes on ceil(max(valid_lens)/128) key chunks: key
positions >= valid_len contribute exactly 0 attention weight (exp of a
large negative bias underflows to 0), so chunks beyond that bound are
skipped entirely.  This is derived from the runtime inputs, so the
kernel stays correct for any valid_lens.
"""

import math
import os
import sys

import numpy as np

for _p in ("/opt/trn_rl_repo",):
    if os.path.isdir(_p) and _p not in sys.path:
        sys.path.insert(0, _p)

import ml_dtypes

D_MODEL = 512
NUM_HEADS = 8
HEAD_DIM = 64
N_CORES = 8
LOCAL_F = 128          # features per core = 2 heads * 64
VAUG = 2 * HEAD_DIM + 2  # 130: [v_h0 (64) | ones | v_h1 (64) | ones]
MASK_NEG = -30000.0

# "bfloat16" or "float32r" (fp32 storage, full-rate matmul w/ reduced mult
# precision) or "float32" (exact, 4x slower matmuls)
DT_NAME = os.environ.get("ATTN_KERNEL_DT", "bfloat16")
TRACE = False

last_results = None  # BassKernelResults of the most recent run (for test.py)

_PROG_CACHE = {}


def _np_dt(name):
    return ml_dtypes.bfloat16 if name == "bfloat16" else np.float32


def _build(nch: int, seq: int, dt_name: str, qk_bias: bool, v_bias: bool):
    from contextlib import ExitStack

    import concourse.bass as bass  # noqa: F401
    import concourse.mybir as mybir
    import concourse.tile as tile
    from concourse import bacc

    DT = getattr(mybir.dt, dt_name)
    F32 = mybir.dt.float32
    EXP = mybir.ActivationFunctionType.Exp
    sk = nch * 128
    n_qt = seq // 512
    n_tp = seq // 1024
    assert seq % 1024 == 0

    nc = bacc.Bacc("TRN2", target_bir_lowering=False, debug=False,
                   num_devices=N_CORES)

    def din(name, shape, dt=DT):
        return nc.dram_tensor(name, shape, dt, kind="ExternalInput").ap()

    xTq = din("xTq", [D_MODEL, seq])
    xTk = din("xTk", [D_MODEL, sk])
    xTv = din("xTv", [D_MODEL, sk])
    # [wqT | wkT | wvT_aug] column blocks
    WQKV = 2 * LOCAL_F + VAUG
    wqkv = din("wqkv", [D_MODEL, WQKV])
    woT = din("woT", [LOCAL_F, D_MODEL])
    # f32 smalls: [bq | bk | bv_aug(VAUG) | maskb(nch)]
    NSM = 2 + VAUG + nch
    smalls_d = din("smalls", [128, NSM], F32)
    out_d = nc.dram_tensor("out", [D_MODEL, seq], F32,
                           kind="ExternalOutput").ap()
    recip_d = nc.dram_tensor("recip_bounce", [2, seq], F32).ap()

    with tile.TileContext(nc) as tc, ExitStack() as ctx:
        const = ctx.enter_context(tc.tile_pool(name="const", bufs=1))

        # ---- stage inputs into SBUF ----
        # smalls + weights on the SWDGE queue (parallel with the big input
        # loads on the sync HWDGE queue); inputs column-split so compute
        # can start before staging completes
        wqkv_sb = const.tile([128, 4, WQKV], DT, tag="wqkv")
        nc.scalar.dma_start(
            out=wqkv_sb, in_=wqkv.rearrange("(c p) f -> p c f", p=128))
        sm_sb = const.tile([128, NSM], F32, tag="sm")
        nc.scalar.dma_start(out=sm_sb, in_=smalls_d)
        wo_sb = const.tile([LOCAL_F, D_MODEL], DT, tag="wo")
        nc.gpsimd.dma_start(out=wo_sb, in_=woT)

        xk_sb = const.tile([128, 4, sk], DT, tag="xk")
        nc.sync.dma_start(out=xk_sb, in_=xTk.rearrange("(c p) s -> p c s", p=128))
        xq_sb = const.tile([128, 4, seq], DT, tag="xq")
        xq_r = xTq.rearrange("(c p) s -> p c s", p=128)
        for j0 in range(0, seq, 512):
            nc.sync.dma_start(out=xq_sb[:, :, j0:j0 + 512],
                              in_=xq_r[:, :, j0:j0 + 512])
        xv_sb = const.tile([128, 4, sk], DT, tag="xv")
        nc.sync.dma_start(out=xv_sb, in_=xTv.rearrange("(c p) s -> p c s", p=128))

        bq_sb = sm_sb[:, 0:1]
        bk_sb = sm_sb[:, 1:2]
        bv_sb = sm_sb[:, 2:2 + VAUG]
        mb_sb = sm_sb[:, 2 + VAUG:2 + VAUG + nch]
        wq_of, wk_of, wv_of = 0, LOCAL_F, 2 * LOCAL_F

        # ---- projections ----
        qT = const.tile([LOCAL_F, seq], DT, tag="qT")
        kT = const.tile([LOCAL_F, sk], DT, tag="kT")
        vaug = const.tile([128, nch, VAUG], DT, tag="vaug")

        with tc.tile_pool(name="ps_p", bufs=3, space="PSUM") as psp:
            def proj_qk(dst, w_of, x_sb, b_sb, j0, width):
                w = min(512, width - j0)
                ps = psp.tile([128, 512], F32, tag="pp",
                              name=f"pp{w_of}_{j0}")
                for dc in range(4):
                    nc.tensor.matmul(
                        ps[:, :w],
                        lhsT=wqkv_sb[:, dc, w_of:w_of + LOCAL_F],
                        rhs=x_sb[:, dc, j0:j0 + w],
                        start=(dc == 0), stop=(dc == 3),
                    )
                nc.vector.tensor_copy(out=dst[:, j0:j0 + w], in_=ps[:, :w])
                if qk_bias:
                    # separate op: TensorScalarPtr has 1 sync-wait slot
                    nc.vector.tensor_scalar_add(
                        out=dst[:, j0:j0 + w], in0=dst[:, j0:j0 + w],
                        scalar1=b_sb)

            # order: k, first q half, v, second q half — matches the order
            # attention consumes them
            for j0 in range(0, sk, 512):
                proj_qk(kT, wk_of, xk_sb, bk_sb, j0, sk)
            for j0 in range(0, seq // 2, 512):
                proj_qk(qT, wq_of, xq_sb, bq_sb, j0, seq)
            nc.vector.memset(vaug[:, :, 64:65], 1.0)
            nc.vector.memset(vaug[:, :, 129:130], 1.0)
            for c in range(nch):
                ps = psp.tile([128, VAUG], F32, tag="ppv")
                for dc in range(4):
                    nc.tensor.matmul(
                        ps,
                        lhsT=xv_sb[:, dc, c * 128:(c + 1) * 128],
                        rhs=wqkv_sb[:, dc, wv_of:wv_of + VAUG],
                        start=(dc == 0), stop=(dc == 3),
                    )
                nc.vector.tensor_copy(out=vaug[:, c, 0:64], in_=ps[:, 0:64])
                nc.vector.tensor_copy(out=vaug[:, c, 65:129], in_=ps[:, 65:129])
                if v_bias:
                    nc.vector.tensor_add(
                        out=vaug[:, c, 0:64], in0=vaug[:, c, 0:64],
                        in1=bv_sb[:, 0:64])
                    nc.vector.tensor_add(
                        out=vaug[:, c, 65:129], in0=vaug[:, c, 65:129],
                        in1=bv_sb[:, 65:129])
            for j0 in range(seq // 2, seq, 512):
                proj_qk(qT, wq_of, xq_sb, bq_sb, j0, seq)

        # ---- attention ----
        stage = const.tile([65, 2, seq], F32, tag="stage")
        rbs = [const.tile([64, seq], F32, tag="rb0", name="rb0"),
               const.tile([64, seq], F32, tag="rb1", name="rb1")]
        cn = const.tile([LOCAL_F, seq], DT, tag="cn")
        with (
            tc.tile_pool(name="ps_s", bufs=2, space="PSUM") as pss,
            tc.tile_pool(name="ps_o", bufs=4, space="PSUM") as pso,
            tc.tile_pool(name="expp", bufs=4) as expp,
        ):
            for tp in range(n_tp):
                q0 = tp * 1024
                oT = {}
                for h in range(2):
                    for t in range(2):
                        oT[h, t] = pso.tile([65, 512], F32, tag="oT", name=f"oT{h}{t}")
                for c in range(nch):
                    scs = []
                    for h in range(2):
                        sc = pss.tile([128, 1024], F32, tag="sc")
                        for t in range(2):
                            nc.tensor.matmul(
                                sc[:, t * 512:(t + 1) * 512],
                                lhsT=kT[h * 64:(h + 1) * 64,
                                        c * 128:(c + 1) * 128],
                                rhs=qT[h * 64:(h + 1) * 64,
                                       q0 + t * 512:q0 + (t + 1) * 512],
                                start=True, stop=True,
                                tile_position=(h * 64, 0),
                            )
                        scs.append(sc)
                    exs = []
                    for h in range(2):
                        ex = expp.tile([128, 1024], DT, tag="ex")
                        nc.scalar.activation(
                            out=ex, in_=scs[h], func=EXP,
                            bias=mb_sb[:, c:c + 1],
                            scale=1.0 / math.sqrt(HEAD_DIM),
                        )
                        exs.append(ex)
                    for h in range(2):
                        for t in range(2):
                            nc.tensor.matmul(
                                oT[h, t],
                                lhsT=vaug[:, c, h * 65:(h + 1) * 65],
                                rhs=exs[h][:, t * 512:(t + 1) * 512],
                                start=(c == 0), stop=(c == nch - 1),
                            )
                for h in range(2):
                    for t in range(2):
                        nc.vector.tensor_copy(
                            out=stage[:, h, q0 + t * 512:q0 + (t + 1) * 512],
                            in_=oT[h, t])
                # ---- normalize this q block (overlaps next tp's attention).
                # Denominators bounce through DRAM to get a partition-
                # broadcast access pattern (SBUF APs need nonzero partition
                # step; DRAM APs don't); reciprocal AFTER the broadcast so
                # all 64 lanes work.
                sl = slice(q0, q0 + 1024)
                nc.sync.dma_start(out=recip_d[:, sl], in_=stage[64:65, :, sl])
                for h in range(2):
                    rb = rbs[h]
                    nc.sync.dma_start(
                        out=rb[:, sl],
                        in_=recip_d[h:h + 1, sl].to_broadcast([64, 1024]))
                    nc.vector.reciprocal_approx_fast(
                        out=rb[:, sl], in_=rb[:, sl])
                    nc.vector.tensor_mul(
                        out=cn[h * 64:(h + 1) * 64, sl],
                        in0=stage[0:64, h, sl], in1=rb[:, sl])

        # ---- output projection (transposed partial) ----
        with (
            tc.tile_pool(name="ps_f", bufs=2, space="PSUM") as psf,
            tc.tile_pool(name="outp", bufs=2) as outp,
        ):
            for odc in range(4):
                ob = outp.tile([128, seq], F32, tag="ob")
                for st in range(n_qt):
                    fp = psf.tile([128, 512], F32, tag="fp")
                    nc.tensor.matmul(
                        fp, lhsT=wo_sb[:, odc * 128:(odc + 1) * 128],
                        rhs=cn[:, st * 512:(st + 1) * 512],
                        start=True, stop=True)
                    if st % 2 == 0:
                        nc.vector.tensor_copy(
                            out=ob[:, st * 512:(st + 1) * 512], in_=fp)
                    else:
                        nc.scalar.copy(
                            out=ob[:, st * 512:(st + 1) * 512], in_=fp)
                nc.sync.dma_start(
                    out=out_d[odc * 128:(odc + 1) * 128, :], in_=ob)

    nc.compile()
    return nc


def kernel(queries, keys, values, valid_lens, Wq, bq, Wk, bk, Wv, bv, Wo, bo):
    global last_results
    queries = np.asarray(queries, dtype=np.float32)
    keys = np.asarray(keys, dtype=np.float32)
    values = np.asarray(values, dtype=np.float32)
    valid_lens = np.asarray(valid_lens).astype(np.int64)
    Wq = np.asarray(Wq, dtype=np.float32)
    Wk = np.asarray(Wk, dtype=np.float32)
    Wv = np.asarray(Wv, dtype=np.float32)
    Wo = np.asarray(Wo, dtype=np.float32)
    bq = np.asarray(bq, dtype=np.float32)
    bk = np.asarray(bk, dtype=np.float32)
    bv = np.asarray(bv, dtype=np.float32)
    bo = np.asarray(bo, dtype=np.float32)

    B, S, D = queries.shape
    assert (B, D) == (2, D_MODEL) and S % 1024 == 0

    Lmax = int(min(max(int(valid_lens.max()), 1), S))
    nch = (Lmax + 127) // 128
    sk = nch * 128

    npdt = _np_dt(DT_NAME)
    qk_bias = bool(np.any(bq) or np.any(bk))
    v_bias = bool(np.any(bv))
    key = (nch, S, DT_NAME, qk_bias, v_bias)
    if key not in _PROG_CACHE:
        _PROG_CACHE[key] = _build(nch, S, DT_NAME, qk_bias, v_bias)
    nc = _PROG_CACHE[key]

    in_maps = []
    for core in range(N_CORES):
        b, hp = divmod(core, 4)
        L = int(valid_lens[b])
        fs = hp * LOCAL_F
        wvT_aug = np.zeros((D, VAUG), np.float32)
        wvT_aug[:, 0:64] = Wv[fs:fs + 64, :].T
        wvT_aug[:, 65:129] = Wv[fs + 64:fs + 128, :].T
        bv_aug = np.zeros((VAUG,), np.float32)
        bv_aug[0:64] = bv[fs:fs + 64]
        bv_aug[64] = 1.0
        bv_aug[65:129] = bv[fs + 64:fs + 128]
        bv_aug[129] = 1.0
        if L == 0:
            mask = np.zeros((sk,), np.float32)  # result discarded on host
        else:
            mask = np.where(np.arange(sk) < L, 0.0, MASK_NEG).astype(np.float32)
        wqkv = np.concatenate(
            [Wq[fs:fs + 128, :].T, Wk[fs:fs + 128, :].T, wvT_aug], axis=1)
        smalls = np.empty((128, 2 + VAUG + nch), np.float32)
        smalls[:, 0] = bq[fs:fs + 128]
        smalls[:, 1] = bk[fs:fs + 128]
        smalls[:, 2:2 + VAUG] = bv_aug
        smalls[:, 2 + VAUG:] = mask.reshape(nch, 128).T
        in_maps.append({
            "xTq": np.ascontiguousarray(queries[b].T).astype(npdt),
            "xTk": np.ascontiguousarray(keys[b, :sk].T).astype(npdt),
            "xTv": np.ascontiguousarray(values[b, :sk].T).astype(npdt),
            "wqkv": np.ascontiguousarray(wqkv).astype(npdt),
            "woT": np.ascontiguousarray(Wo[:, fs:fs + 128].T).astype(npdt),
            "smalls": smalls,
        })

    from concourse.bass_utils import run_bass_kernel_spmd
    res = run_bass_kernel_spmd(nc, in_maps, list(range(N_CORES)), trace=TRACE)
    last_results = res
    outs = [r["out"] for r in res.results]

    final = np.empty((B, S, D), np.float32)
    for b in range(B):
        acc = outs[4 * b] + outs[4 * b + 1] + outs[4 * b + 2] + outs[4 * b + 3]
        final[b] = acc.T + bo
        if int(valid_lens[b]) == 0:
            # uniform attention over all S positions (reference semantics
            # when every key is masked: softmax of a constant row)
            row = (values[b].mean(0) @ Wv.T + bv) @ Wo.T + bo
            final[b] = np.broadcast_to(row, (S, D))
    return final


# revision 28
# speedup vs baseline: 1.7098x; 1.2736x over previous
"""Trainium2 Bass kernel: masked multi-head attention (B=2, S=2048, D=512, H=8).

Sharding: batch x head-pair across 8 cores (core = b*4 + head_pair).
Each core computes, for its batch b and its 2 heads:
    q/k/v projections -> scores^T -> exp (mask folded in as per-partition
    bias on the ScalarE) -> attn@v with a ones-column appended to V (gives
    the softmax denominator for free) -> normalize -> partial out-proj.
The 4 per-batch partials are summed on the host (the "all-reduce"), then
bias bo is added.

Device layouts (per core):
  xTq/xTk/xTv  [D, S]    inputs pre-transposed on host (feature-major)
  q/k projT    [128, S]  2 local heads stacked on partitions (h0: 0-63)
  scores^T     [128k, q] per 128-wide key chunk; softmax mask depends only
                         on the key position -> per-partition ACT bias
  v_aug        [Sk, 130] per-head [Wv_h | ones] columns; attn@v output row
                         64 of each head block is the softmax denominator
  out          [512, S]  transposed partial output (host transposes back)

The kernel specializes on ceil(max(valid_lens)/128) key chunks: key
positions >= valid_len contribute exactly 0 attention weight (exp of a
large negative bias underflows to 0), so chunks beyond that bound are
skipped entirely.  This is derived from the runtime inputs, so the
kernel stays correct for any valid_lens.
"""

import math
import os
import sys

import numpy as np

for _p in ("/opt/trn_rl_repo",):
    if os.path.isdir(_p) and _p not in sys.path:
        sys.path.insert(0, _p)

import ml_dtypes

D_MODEL = 512
NUM_HEADS = 8
HEAD_DIM = 64
N_CORES = 8
LOCAL_F = 128          # features per core = 2 heads * 64
VAUG = 2 * HEAD_DIM + 2  # 130: [v_h0 (64) | ones | v_h1 (64) | ones]
MASK_NEG = -30000.0

# "bfloat16" or "float32r" (fp32 storage, full-rate matmul w/ reduced mult
# precision) or "float32" (exact, 4x slower matmuls)
DT_NAME = os.environ.get("ATTN_KERNEL_DT", "bfloat16")
TRACE = False

last_results = None  # BassKernelResults of the most recent run (for test.py)

_PROG_CACHE = {}


def _np_dt(name):
    return ml_dtypes.bfloat16 if name == "bfloat16" else np.float32


def _build(nch: int, seq: int, dt_name: str, qk_bias: bool, v_bias: bool):
    from contextlib import ExitStack

    import concourse.bass as bass  # noqa: F401
    import concourse.mybir as mybir
    import concourse.tile as tile
    from concourse import bacc

    DT = getattr(mybir.dt, dt_name)
    F32 = mybir.dt.float32
    EXP = mybir.ActivationFunctionType.Exp
    sk = nch * 128
    n_qt = seq // 512
    n_tp = seq // 1024
    assert seq % 1024 == 0

    nc = bacc.Bacc("TRN2", target_bir_lowering=False, debug=False,
                   num_devices=N_CORES)

    def din(name, shape, dt=DT):
        return nc.dram_tensor(name, shape, dt, kind="ExternalInput").ap()

    xTq = din("xTq", [D_MODEL, seq])
    xTk = din("xTk", [D_MODEL, sk])
    xTv = din("xTv", [D_MODEL, sk])
    # [wqT | wkT | wvT_aug] column blocks
    WQKV = 2 * LOCAL_F + VAUG
    wqkv = din("wqkv", [D_MODEL, WQKV])
    woT = din("woT", [LOCAL_F, D_MODEL])
    # f32 smalls: [bq | bk | bv_aug(VAUG) | maskb(nch)]
    NSM = 2 + VAUG + nch
    smalls_d = din("smalls", [128, NSM], F32)
    out_d = nc.dram_tensor("out", [D_MODEL, seq], F32,
                           kind="ExternalOutput").ap()
    recip_d = nc.dram_tensor("recip_bounce", [2, seq], F32).ap()

    with tile.TileContext(nc) as tc, ExitStack() as ctx:
        const = ctx.enter_context(tc.tile_pool(name="const", bufs=1))

        # ---- stage inputs into SBUF ----
        # smalls + weights on the SWDGE queue (parallel with the big input
        # loads on the sync HWDGE queue); inputs column-split so compute
        # can start before staging completes
        wqkv_sb = const.tile([128, 4, WQKV], DT, tag="wqkv")
        nc.scalar.dma_start(
            out=wqkv_sb, in_=wqkv.rearrange("(c p) f -> p c f", p=128))
        sm_sb = const.tile([128, NSM], F32, tag="sm")
        nc.scalar.dma_start(out=sm_sb, in_=smalls_d)
        wo_sb = const.tile([LOCAL_F, D_MODEL], DT, tag="wo")
        nc.gpsimd.dma_start(out=wo_sb, in_=woT)

        xk_sb = const.tile([128, 4, sk], DT, tag="xk")
        nc.sync.dma_start(out=xk_sb, in_=xTk.rearrange("(c p) s -> p c s", p=128))
        xq_sb = const.tile([128, 4, seq], DT, tag="xq")
        xq_r = xTq.rearrange("(c p) s -> p c s", p=128)
        for j0 in range(0, seq, 512):
            nc.sync.dma_start(out=xq_sb[:, :, j0:j0 + 512],
                              in_=xq_r[:, :, j0:j0 + 512])
        xv_sb = const.tile([128, 4, sk], DT, tag="xv")
        nc.sync.dma_start(out=xv_sb, in_=xTv.rearrange("(c p) s -> p c s", p=128))

        bq_sb = sm_sb[:, 0:1]
        bk_sb = sm_sb[:, 1:2]
        bv_sb = sm_sb[:, 2:2 + VAUG]
        mb_sb = sm_sb[:, 2 + VAUG:2 + VAUG + nch]
        wq_of, wk_of, wv_of = 0, LOCAL_F, 2 * LOCAL_F

        # ---- projections ----
        qT = const.tile([LOCAL_F, seq], DT, tag="qT")
        kT = const.tile([LOCAL_F, sk], DT, tag="kT")
        vaug = const.tile([128, nch, VAUG], DT, tag="vaug")

        with tc.tile_pool(name="ps_p", bufs=3, space="PSUM") as psp:
            def proj_qk(dst, w_of, x_sb, b_sb, j0, width):
                w = min(512, width - j0)
                ps = psp.tile([128, 512], F32, tag="pp",
                              name=f"pp{w_of}_{j0}")
                for dc in range(4):
                    nc.tensor.matmul(
                        ps[:, :w],
                        lhsT=wqkv_sb[:, dc, w_of:w_of + LOCAL_F],
                        rhs=x_sb[:, dc, j0:j0 + w],
                        start=(dc == 0), stop=(dc == 3),
                    )
                nc.vector.tensor_copy(out=dst[:, j0:j0 + w], in_=ps[:, :w])
                if qk_bias:
                    # separate op: TensorScalarPtr has 1 sync-wait slot
                    nc.vector.tensor_scalar_add(
                        out=dst[:, j0:j0 + w], in0=dst[:, j0:j0 + w],
                        scalar1=b_sb)

            # order: k, first q half, v, second q half — matches the order
            # attention consumes them
            for j0 in range(0, sk, 512):
                proj_qk(kT, wk_of, xk_sb, bk_sb, j0, sk)
            for j0 in range(0, seq // 2, 512):
                proj_qk(qT, wq_of, xq_sb, bq_sb, j0, seq)
            nc.vector.memset(vaug[:, :, 64:65], 1.0)
            nc.vector.memset(vaug[:, :, 129:130], 1.0)
            for c in range(nch):
                ps = psp.tile([128, VAUG], F32, tag="ppv")
                for dc in range(4):
                    nc.tensor.matmul(
                        ps,
                        lhsT=xv_sb[:, dc, c * 128:(c + 1) * 128],
                        rhs=wqkv_sb[:, dc, wv_of:wv_of + VAUG],
                        start=(dc == 0), stop=(dc == 3),
                    )
                nc.vector.tensor_copy(out=vaug[:, c, 0:64], in_=ps[:, 0:64])
                nc.vector.tensor_copy(out=vaug[:, c, 65:129], in_=ps[:, 65:129])
                if v_bias:
                    nc.vector.tensor_add(
                        out=vaug[:, c, 0:64], in0=vaug[:, c, 0:64],
                        in1=bv_sb[:, 0:64])
                    nc.vector.tensor_add(
                        out=vaug[:, c, 65:129], in0=vaug[:, c, 65:129],
                        in1=bv_sb[:, 65:129])
            for j0 in range(seq // 2, seq, 512):
                proj_qk(qT, wq_of, xq_sb, bq_sb, j0, seq)

        # ---- attention ----
        stage = const.tile([65, 2, seq], F32, tag="stage")
        rbs = [const.tile([64, seq], F32, tag="rb0", name="rb0"),
               const.tile([64, seq], F32, tag="rb1", name="rb1")]
        cn = const.tile([LOCAL_F, seq], DT, tag="cn")
        with (
            tc.tile_pool(name="ps_s", bufs=3, space="PSUM") as pss,
            tc.tile_pool(name="ps_o", bufs=2, space="PSUM") as pso,
            tc.tile_pool(name="expp", bufs=4) as expp,
        ):
            for tp in range(n_tp):
                q0 = tp * 1024

                for h in range(2):
                    oT = {t: pso.tile([65, 512], F32, tag="oT",
                                      name=f"oT{tp}{h}{t}") for t in range(2)}
                    exs = [None] * nch

                    def attn_v(c):
                        for t in range(2):
                            nc.tensor.matmul(
                                oT[t],
                                lhsT=vaug[:, c, h * 65:(h + 1) * 65],
                                rhs=exs[c][:, t * 512:(t + 1) * 512],
                                start=(c == 0), stop=(c == nch - 1),
                            )

                    for c in range(nch):
                        sc = pss.tile([128, 1024], F32, tag="sc")
                        for t in range(2):
                            nc.tensor.matmul(
                                sc[:, t * 512:(t + 1) * 512],
                                lhsT=kT[h * 64:(h + 1) * 64,
                                        c * 128:(c + 1) * 128],
                                rhs=qT[h * 64:(h + 1) * 64,
                                       q0 + t * 512:q0 + (t + 1) * 512],
                                start=True, stop=True,
                            )
                        ex = expp.tile([128, 1024], DT, tag="ex")
                        nc.scalar.activation(
                            out=ex, in_=sc, func=EXP,
                            bias=mb_sb[:, c:c + 1],
                            scale=1.0 / math.sqrt(HEAD_DIM),
                        )
                        exs[c] = ex
                        # attn@v lags one chunk so the PE never waits on exp
                        if c > 0:
                            attn_v(c - 1)
                    attn_v(nch - 1)
                    for t in range(2):
                        nc.vector.tensor_copy(
                            out=stage[:, h, q0 + t * 512:q0 + (t + 1) * 512],
                            in_=oT[t])
                # ---- normalize this q block (overlaps next tp's attention).
                # Denominators bounce through DRAM to get a partition-
                # broadcast access pattern (SBUF APs need nonzero partition
                # step; DRAM APs don't); reciprocal AFTER the broadcast so
                # all 64 lanes work.
                sl = slice(q0, q0 + 1024)
                nc.sync.dma_start(out=recip_d[:, sl], in_=stage[64:65, :, sl])
                for h in range(2):
                    rb = rbs[h]
                    nc.sync.dma_start(
                        out=rb[:, sl],
                        in_=recip_d[h:h + 1, sl].to_broadcast([64, 1024]))
                    nc.vector.reciprocal_approx_fast(
                        out=rb[:, sl], in_=rb[:, sl])
                    nc.vector.tensor_mul(
                        out=cn[h * 64:(h + 1) * 64, sl],
                        in0=stage[0:64, h, sl], in1=rb[:, sl])

        # ---- output projection (transposed partial) ----
        with (
            tc.tile_pool(name="ps_f", bufs=2, space="PSUM") as psf,
            tc.tile_pool(name="outp", bufs=2) as outp,
        ):
            for odc in range(4):
                ob = outp.tile([128, seq], F32, tag="ob")
                for st in range(n_qt):
                    fp = psf.tile([128, 512], F32, tag="fp")
                    nc.tensor.matmul(
                        fp, lhsT=wo_sb[:, odc * 128:(odc + 1) * 128],
                        rhs=cn[:, st * 512:(st + 1) * 512],
                        start=True, stop=True)
                    if st % 2 == 0:
                        nc.vector.tensor_copy(
                            out=ob[:, st * 512:(st + 1) * 512], in_=fp)
                    else:
                        nc.scalar.copy(
                            out=ob[:, st * 512:(st + 1) * 512], in_=fp)
                nc.sync.dma_start(
                    out=out_d[odc * 128:(odc + 1) * 128, :], in_=ob)

    nc.compile()
    return nc


def kernel(queries, keys, values, valid_lens, Wq, bq, Wk, bk, Wv, bv, Wo, bo):
    global last_results
    queries = np.asarray(queries, dtype=np.float32)
    keys = np.asarray(keys, dtype=np.float32)
    values = np.asarray(values, dtype=np.float32)
    valid_lens = np.asarray(valid_lens).astype(np.int64)
    Wq = np.asarray(Wq, dtype=np.float32)
    Wk = np.asarray(Wk, dtype=np.float32)
    Wv = np.asarray(Wv, dtype=np.float32)
    Wo = np.asarray(Wo, dtype=np.float32)
    bq = np.asarray(bq, dtype=np.float32)
    bk = np.asarray(bk, dtype=np.float32)
    bv = np.asarray(bv, dtype=np.float32)
    bo = np.asarray(bo, dtype=np.float32)

    B, S, D = queries.shape
    assert (B, D) == (2, D_MODEL) and S % 1024 == 0

    Lmax = int(min(max(int(valid_lens.max()), 1), S))
    nch = (Lmax + 127) // 128
    sk = nch * 128

    npdt = _np_dt(DT_NAME)
    qk_bias = bool(np.any(bq) or np.any(bk))
    v_bias = bool(np.any(bv))
    key = (nch, S, DT_NAME, qk_bias, v_bias)
    if key not in _PROG_CACHE:
        _PROG_CACHE[key] = _build(nch, S, DT_NAME, qk_bias, v_bias)
    nc = _PROG_CACHE[key]

    in_maps = []
    for core in range(N_CORES):
        b, hp = divmod(core, 4)
        L = int(valid_lens[b])
        fs = hp * LOCAL_F
        wvT_aug = np.zeros((D, VAUG), np.float32)
        wvT_aug[:, 0:64] = Wv[fs:fs + 64, :].T
        wvT_aug[:, 65:129] = Wv[fs + 64:fs + 128, :].T
        bv_aug = np.zeros((VAUG,), np.float32)
        bv_aug[0:64] = bv[fs:fs + 64]
        bv_aug[64] = 1.0
        bv_aug[65:129] = bv[fs + 64:fs + 128]
        bv_aug[129] = 1.0
        if L == 0:
            mask = np.zeros((sk,), np.float32)  # result discarded on host
        else:
            mask = np.where(np.arange(sk) < L, 0.0, MASK_NEG).astype(np.float32)
        wqkv = np.concatenate(
            [Wq[fs:fs + 128, :].T, Wk[fs:fs + 128, :].T, wvT_aug], axis=1)
        smalls = np.empty((128, 2 + VAUG + nch), np.float32)
        smalls[:, 0] = bq[fs:fs + 128]
        smalls[:, 1] = bk[fs:fs + 128]
        smalls[:, 2:2 + VAUG] = bv_aug
        smalls[:, 2 + VAUG:] = mask.reshape(nch, 128).T
        in_maps.append({
            "xTq": np.ascontiguousarray(queries[b].T).astype(npdt),
            "xTk": np.ascontiguousarray(keys[b, :sk].T).astype(npdt),
            "xTv": np.ascontiguousarray(values[b, :sk].T).astype(npdt),
            "wqkv": np.ascontiguousarray(wqkv).astype(npdt),
            "woT": np.ascontiguousarray(Wo[:, fs:fs + 128].T).astype(npdt),
            "smalls": smalls,
        })

    from concourse.bass_utils import run_bass_kernel_spmd
    res = run_bass_kernel_spmd(nc, in_maps, list(range(N_CORES)), trace=TRACE)
    last_results = res
    outs = [r["out"] for r in res.results]

    final = np.empty((B, S, D), np.float32)
    for b in range(B):
        acc = outs[4 * b] + outs[4 * b + 1] + outs[4 * b + 2] + outs[4 * b + 3]
        final[b] = acc.T + bo
        if int(valid_lens[b]) == 0:
            # uniform attention over all S positions (reference semantics
            # when every key is masked: softmax of a constant row)
            row = (values[b].mean(0) @ Wv.T + bv) @ Wo.T + bo
            final[b] = np.broadcast_to(row, (S, D))
    return final
